# revision 1
# baseline (speedup 1.0000x reference)
"""Trainium2 Bass kernel for nn_Attention1x1 (channel attention with 1x1 convs).

Math (per sample b):
  qkv = (w_qkv * w_dw[:,None]) @ x          x: [C, N]  (N = H*W)
  q, k, v = split(qkv)
  attn = softmax( (q_n @ k_n^T) * temp ),   q_n/k_n L2-normalized over N
  out = w_proj @ (attn @ v)

Key identity used: with Wq/Wk/Wv the dw-folded weight blocks,
  q @ k^T   = Wq Gx Wk^T        where Gx = x @ x^T   [C, C]
  ||q_c||^2 = diag(Wq Gx Wq^T),  ||k_d||^2 = diag(Wk Gx Wk^T)
  out       = (Wproj @ attn @ Wv) @ x = W2 @ x
So only two big (N-sized) matmuls are needed: the Gram of x and the final
projection W2 @ x.  Everything else is 256x256.

Sharding: data-parallel over batch, one sample per NeuronCore (B=8, 8 cores).

Per-core pipeline:
  Stage A: DMA x into SBUF (kept resident), per 128-col chunk: cast to bf16,
           PE-transpose, accumulate Gx in PSUM (bf16 matmuls, fp32 accum).
           bf16 is safe here: logits are cosines ~1e-2, softmax flattens them.
  Stage B: small 256x256 matmuls (fp32) for S = Wq Gx Wk^T, row/col norms via
           elementwise mult + ones-matmul, softmax, W2T = Wv^T A^T Wproj^T.
  Stage C: out = W2 @ x from SBUF-resident x, float32r matmuls (11-bit
           mantissa rounding, ~2.4e-4 rel err), DMA out.
"""

import sys
import numpy as np

if "/opt/trn_rl_repo" not in sys.path:
    sys.path.insert(0, "/opt/trn_rl_repo")

B, C, H, W = 8, 256, 128, 128
N = H * W
EPS2 = 1e-24  # max(||q||, 1e-12) applied on the squared norm

_CACHE = {}


def _build(n, reps=1, compile=True):
    from contextlib import ExitStack
    import concourse.bass as bass
    import concourse.bacc as bacc
    import concourse.tile as tile
    from concourse import mybir, masks

    f32 = mybir.dt.float32
    f32r = mybir.dt.float32r
    bf16 = mybir.dt.bfloat16
    AX = mybir.AxisListType
    AF = mybir.ActivationFunctionType

    nc = bacc.Bacc("TRN2", target_bir_lowering=False, debug=False)

    x_d = nc.dram_tensor("x", [C, n], f32r, kind="ExternalInput")
    wqkT_d = nc.dram_tensor("wqkT", [C, 2 * C], f32r, kind="ExternalInput")
    wv_d = nc.dram_tensor("wv", [C, C], f32r, kind="ExternalInput")
    wprojT_d = nc.dram_tensor("wprojT", [C, C], f32, kind="ExternalInput")
    temp_d = nc.dram_tensor("temp", [1, 1], f32, kind="ExternalInput")
    out_d = nc.dram_tensor("out", [C, n], f32, kind="ExternalOutput")

    n_chunks = n // 128  # stage A chunk count
    n2_chunks = n // 512  # stage C chunk count

    with tile.TileContext(nc) as tc, ExitStack() as ctx:
        # ---- persistent SBUF ----
        persist = ctx.enter_context(tc.tile_pool(name="persist", bufs=1))
        xbuf = [persist.tile([128, n], f32r, tag=f"xbuf{k}", name=f"xbuf{k}") for k in range(2)]
        wqkT_sb = [persist.tile([128, 2 * C], f32r, tag=f"wqkT{k}", name=f"wqkT{k}") for k in range(2)]
        wv_sb = [persist.tile([128, C], f32r, tag=f"wv{k}", name=f"wv{k}") for k in range(2)]
        wprojT_sb = [persist.tile([128, C], f32, tag=f"wprojT{k}", name=f"wprojT{k}") for k in range(2)]
        temp_sb = persist.tile([1, 1], f32, tag="temp", name="temp")
        ident = persist.tile([128, 128], bf16, tag="ident", name="ident")
        ones_col = persist.tile([128, 1], f32r, tag="ones_col", name="ones_col")
        ones_col_f = persist.tile([128, 1], f32, tag="ones_col_f", name="ones_col_f")
        ones_row = persist.tile([1, 128], f32, tag="ones_row", name="ones_row")
        one_11 = persist.tile([1, 1], f32, tag="one_11", name="one_11")

        masks.make_identity(nc, ident[:])
        nc.gpsimd.memset(ones_col_f[:], 1.0)
        nc.scalar.copy(ones_col[:], ones_col_f[:])
        # preload ACT LUTs (Exp/Sqrt) so stage-B doesn't stall on table DMAs
        actwarm = persist.tile([128, 1], f32, tag="actwarm", name="actwarm")
        nc.scalar.activation(actwarm[:], ones_col_f[:], AF.Exp)
        nc.scalar.activation(actwarm[:], ones_col_f[:], AF.Sqrt)
        nc.gpsimd.memset(ones_row[:], 1.0)
        nc.gpsimd.memset(one_11[:], 1.0)

        for k in range(2):
            nc.sync.dma_start(wqkT_sb[k][:], wqkT_d[128 * k : 128 * (k + 1), :])
            nc.sync.dma_start(wv_sb[k][:], wv_d[128 * k : 128 * (k + 1), :])
            nc.sync.dma_start(wprojT_sb[k][:], wprojT_d[128 * k : 128 * (k + 1), :])
        nc.sync.dma_start(temp_sb[:], temp_d[:])

        # ================= Stage A: load x, Gram of x (bf16) =================
        gx_pool = ctx.enter_context(tc.tile_pool(name="gx_ps", bufs=1, space="PSUM"))
        gx_ps = [gx_pool.tile([128, C], f32, tag=f"gx{m}", name=f"gx{m}") for m in range(2)]
        small = ctx.enter_context(tc.tile_pool(name="small", bufs=1))

        for _rep in range(reps):
            with tc.tile_pool(name="stageA", bufs=6) as apool, tc.tile_pool(
                name="pt_ps", bufs=6, space="PSUM"
            ) as ptpool:
                for jp in range(n_chunks // 2):
                    n0 = 256 * jp  # first chunk of this pair
                    if n0 % 2048 == 0:  # 1 MiB DMA granularity per x row-tile
                        if n0 == n - 2048:
                            # last block: fine-grained pieces -> short pipeline tail
                            for q in range(4):
                                for k in range(2):
                                    nc.sync.dma_start(
                                        xbuf[k][:, n0 + 512 * q : n0 + 512 * (q + 1)],
                                        x_d[
                                            128 * k : 128 * (k + 1),
                                            n0 + 512 * q : n0 + 512 * (q + 1),
                                        ],
                                    )
                        else:
                            for k in range(2):
                                nc.sync.dma_start(
                                    xbuf[k][:, n0 : n0 + 2048],
                                    x_d[128 * k : 128 * (k + 1), n0 : n0 + 2048],
                                )
                    # cast fp32 -> bf16 (DVE, 512 cols at a time)
                    if n0 % 512 == 0:
                        xb = [
                            apool.tile([128, 512], bf16, tag=f"xb{k}", name=f"xb{k}")
                            for k in range(2)
                        ]
                        for k in range(2):
                            nc.vector.tensor_copy(
                                xb[k][:], xbuf[k][:, n0 : n0 + 512].bitcast(f32)
                            )
                    # 4 PE-transposes into one PSUM tile, one evac per pair
                    pt = ptpool.tile([128, 2 * C], bf16, tag="pt", name="pt")
                    for sub in range(2):
                        jj = (2 * jp + sub) % 4
                        for k in range(2):
                            nc.tensor.transpose(
                                pt[:, 256 * sub + 128 * k : 256 * sub + 128 * (k + 1)],
                                xb[k][:, 128 * jj : 128 * (jj + 1)],
                                ident[:],
                            )
                    xt = apool.tile([128, 2 * C], bf16, tag="xt", name="xt")
                    if jp % 2 == 0:
                        nc.scalar.copy(xt[:], pt[:])
                    else:
                        nc.vector.tensor_copy(xt[:], pt[:])
                    for sub in range(2):
                        for m in range(2):
                            nc.tensor.matmul(
                                gx_ps[m][:],
                                xt[:, 256 * sub + 128 * m : 256 * sub + 128 * (m + 1)],
                                xt[:, 256 * sub : 256 * sub + 256],
                                start=(jp == 0 and sub == 0),
                                stop=(jp == n_chunks // 2 - 1 and sub == 1),
                                skip_group_check=True,
                            )

            # ================= Stage B: attention smalls (fp32) =================
            with tc.tile_pool(name="psB", bufs=6, space="PSUM") as psB:
                gx_sb = [small.tile([128, C], f32r, tag=f"gx_sb{m}", name=f"gx_sb{m}") for m in range(2)]
                nc.scalar.copy(gx_sb[0][:], gx_ps[0][:])
                nc.vector.tensor_copy(gx_sb[1][:], gx_ps[1][:])

                # UV = Gx @ [WqT | WkT]  -> [C, 2C]
                uv_ps = [psB.tile([128, 2 * C], f32, tag="ps", name=f"uv{m}") for m in range(2)]
                for m in range(2):
                    for k in range(2):
                        nc.tensor.matmul(
                            uv_ps[m][:],
                            gx_sb[k][:, 128 * m : 128 * (m + 1)],
                            wqkT_sb[k][:],
                            start=(k == 0),
                            stop=(k == 1),
                        )
                uv_sb = [small.tile([128, 2 * C], f32r, tag=f"uv_sb{m}", name=f"uv_sb{m}") for m in range(2)]
                nc.scalar.copy(uv_sb[0][:], uv_ps[0][:])
                nc.vector.tensor_copy(uv_sb[1][:], uv_ps[1][:])

                # S = Wq Gx Wk^T = WqT^T @ V   -> [C, C]
                s_ps = [psB.tile([128, C], f32, tag="ps", name=f"s{m}") for m in range(2)]
                for m in range(2):
                    for k in range(2):
                        nc.tensor.matmul(
                            s_ps[m][:],
                            wqkT_sb[k][:, 128 * m : 128 * (m + 1)],
                            uv_sb[k][:, C : 2 * C],
                            start=(k == 0),
                            stop=(k == 1),
                        )

                # Nq | Nk = colsum(WqkT . UV)  -> [1, 2C]
                pr = [small.tile([128, 2 * C], f32r, tag=f"pr{k}", name=f"pr{k}") for k in range(2)]
                for k in range(2):
                    nc.vector.tensor_mul(
                        pr[k][:], wqkT_sb[k][:].bitcast(f32), uv_ps[k][:]
                    )
                nqk_ps = psB.tile([1, 2 * C], f32, tag="ps", name="nqk")
                for k in range(2):
                    nc.tensor.matmul(
                        nqk_ps[:], ones_col[:], pr[k][:], start=(k == 0), stop=(k == 1)
                    )

                # inv = temp-ish / max(sqrt(nqk), eps): rsqrt(max(nqk, eps^2))
                inv = small.tile([1, 2 * C], f32, tag="inv", name="inv")
                nc.vector.tensor_scalar_max(inv[:], nqk_ps[:], EPS2)
                nc.scalar.activation(inv[:], inv[:], AF.Sqrt)
                nc.vector.reciprocal(inv[:], inv[:])
                # fold temperature into the q-side scale
                nc.vector.tensor_scalar_mul(
                    inv[:, 0:C], inv[:, 0:C], temp_sb[0:1, 0:1]
                )

                # column vector forms of inv_nq (per-partition scalars)
                invq_ps = psB.tile([128, 2], f32, tag="ps", name="invq")
                for m in range(2):
                    nc.tensor.matmul(
                        invq_ps[:, m : m + 1],
                        inv[0:1, 128 * m : 128 * (m + 1)],
                        one_11[:],
                        start=True,
                        stop=True,
                        skip_group_check=True,
                    )
                invq_sb = small.tile([128, 2], f32, tag="invq_sb", name="invq_sb")
                nc.scalar.copy(invq_sb[:], invq_ps[:])

                # broadcast of inv_nk across partitions: ones_row^T @ inv_nk
                nkb_ps = psB.tile([128, C], f32, tag="ps", name="nkb")
                nc.tensor.matmul(
                    nkb_ps[:], ones_row[:], inv[0:1, C : 2 * C], start=True, stop=True
                )
                nkb_sb = small.tile([128, C], f32, tag="nkb_sb", name="nkb_sb")
                nc.scalar.copy(nkb_sb[:], nkb_ps[:])

                # logits L = S * inv_nq[c] * inv_nk[d]; softmax rows -> A
                # E = exp(L - max); softmax denominator folded into WprojT rows
                e_sb = [small.tile([128, C], f32r, tag=f"e{m}", name=f"e{m}") for m in range(2)]
                wps = [small.tile([128, C], f32r, tag=f"wps{m}", name=f"wps{m}") for m in range(2)]
                for m in range(2):
                    L = small.tile([128, C], f32, tag="L", name="L")
                    nc.vector.scalar_tensor_tensor(
                        L[:],
                        s_ps[m][:],
                        invq_sb[:, m : m + 1],
                        nkb_sb[:],
                        op0=mybir.AluOpType.mult,
                        op1=mybir.AluOpType.mult,
                    )
                    rsum = small.tile([128, 1], f32, tag="rsum", name="rsum")
                    nc.scalar.activation(
                        e_sb[m][:], L[:], AF.Exp, accum_out=rsum[:]
                    )
                    rinv = small.tile([128, 1], f32, tag="rinv", name="rinv")
                    nc.vector.reciprocal(rinv[:], rsum[:])
                    nc.vector.tensor_scalar_mul(wps[m][:], wprojT_sb[m][:], rinv[:])

                # R1 = A^T @ WprojT  -> [C, C]
                r1_ps = [psB.tile([128, C], f32, tag="ps", name=f"r1{m}") for m in range(2)]
                for m in range(2):
                    for k in range(2):
                        nc.tensor.matmul(
                            r1_ps[m][:],
                            e_sb[k][:, 128 * m : 128 * (m + 1)],
                            wps[k][:],
                            start=(k == 0),
                            stop=(k == 1),
                        )
                r1_sb = [small.tile([128, C], f32r, tag=f"r1_sb{m}", name=f"r1_sb{m}") for m in range(2)]
                nc.scalar.copy(r1_sb[0][:], r1_ps[0][:])
                nc.vector.tensor_copy(r1_sb[1][:], r1_ps[1][:])

                # W2T = Wv^T @ R1  -> [C, C], rounded to f32r on evacuation
                w2_ps = [psB.tile([128, C], f32, tag="ps", name=f"w2{m}") for m in range(2)]
                for m in range(2):
                    for k in range(2):
                        nc.tensor.matmul(
                            w2_ps[m][:],
                            wv_sb[k][:, 128 * m : 128 * (m + 1)],
                            r1_sb[k][:],
                            start=(k == 0),
                            stop=(k == 1),
                        )
                w2t_sb = [small.tile([128, C], f32r, tag=f"w2t{m}", name=f"w2t{m}") for m in range(2)]
                nc.scalar.copy(w2t_sb[0][:], w2_ps[0][:])
                nc.vector.tensor_copy(w2t_sb[1][:], w2_ps[1][:])

            # ================= Stage C: out = W2 @ x (f32r) =================
            with tc.tile_pool(name="stageC", bufs=4) as cpool, tc.tile_pool(
                name="psC", bufs=3, space="PSUM"
            ) as psC:
                for j in range(n2_chunks):
                    n0 = 512 * j
                    op = psC.tile([128, 1024], f32, tag="op", name="op")
                    for m in range(2):
                        for k in range(2):
                            nc.tensor.matmul(
                                op[:, 512 * m : 512 * (m + 1)],
                                w2t_sb[k][:, 128 * m : 128 * (m + 1)],
                                xbuf[k][:, n0 : n0 + 512],
                                start=(k == 0),
                                stop=(k == 1),
                                skip_group_check=True,
                            )
                    osb = cpool.tile([128, 1024], f32, tag="osb", name="osb")
                    nc.scalar.copy(osb[:], op[:])
                    for m in range(2):
                        nc.sync.dma_start(
                            out_d[128 * m : 128 * (m + 1), n0 : n0 + 512],
                            osb[:, 512 * m : 512 * (m + 1)],
                        )

    if compile:
        nc.compile()
    return nc


def _get_nc(n=N, reps=1):
    key = ("nc", n, reps)
    if key not in _CACHE:
        _CACHE[key] = _build(n, reps)
    return _CACHE[key]


def kernel(x, w_qkv, w_dw, temperature, w_proj):
    from concourse.bass_utils import run_bass_kernel_spmd

    x = np.ascontiguousarray(np.asarray(x, dtype=np.float32))
    w_qkv = np.asarray(w_qkv, dtype=np.float32)
    w_dw = np.asarray(w_dw, dtype=np.float32)
    w_proj = np.asarray(w_proj, dtype=np.float32)
    b, c, h, w = x.shape
    n = h * w

    wf = w_qkv * w_dw[:, None]
    wqkT = np.ascontiguousarray(wf[: 2 * c].T)        # [C, 2C] = [WqT | WkT]
    wv = np.ascontiguousarray(wf[2 * c : 3 * c])      # [C, C] native [d, i]
    wprojT = np.ascontiguousarray(w_proj.T)           # [C, C] = [c, p]
    temp = np.asarray(temperature, dtype=np.float32).reshape(1, 1)

    nc = _get_nc(n)
    in_maps = [
        {
            "x": x[i].reshape(c, n),
            "wqkT": wqkT,
            "wv": wv,
            "wprojT": wprojT,
            "temp": temp,
        }
        for i in range(b)
    ]
    res = run_bass_kernel_spmd(nc, in_maps, list(range(b)))
    out = np.stack([res.results[i]["out"].reshape(c, h, w) for i in range(b)])
    return out.astype(np.float32)


if __name__ == "__main__":
    rng = np.random.default_rng(0)
    x = rng.standard_normal((B, C, H, W), dtype=np.float32)
    w_qkv = (rng.standard_normal((3 * C, C)) * 0.02).astype(np.float32)
    w_dw = (rng.standard_normal(3 * C) * 0.1 + 1.0).astype(np.float32)
    temperature = np.ones((1, 1, 1), np.float32)
    w_proj = (rng.standard_normal((C, C)) * 0.02).astype(np.float32)
    out = kernel(x=x, w_qkv=w_qkv, w_dw=w_dw, temperature=temperature, w_proj=w_proj)
    print("out", out.shape, out.dtype, float(np.abs(out).max()))

